# revision 27
# baseline (speedup 1.0000x reference)
"""Bass/Trainium2 kernel for nn_Attn (Bahdanau 'general' attention scoring).

Reference math:
    energies = einsum('sd,hd,h->s', enc, W, hidden) + b.hidden
    out      = softmax(energies)[None, None, :]

Factorization:
    v = W^T @ hidden (200-dim), energies = enc @ v (+ const; softmax cancels
    the constant b.hidden term, so b is dropped).

Distribution (8 NeuronCores, one TRN2 chip) — d-sharding: core i owns
d-slice [25*i, 25*(i+1)) of the contraction dim:
    W slice  [8192, 25]  -> v_i = W_i^T @ hidden (exact, local, no comm)
    enc slice [32768, 25] -> partial energies e_i[s] = enc[s, d_i] . v_i
for ALL 32768 positions, laid out [128, 256] (s = p*256 + f), then ONE
AllReduce(add) over the 128KB partials. Rationale from warmed profiles:
  - The collective entry barrier + first-collective setup complete by
    ~50us (global) when a dependency-free warm-up AllGather rings the
    doorbell right after the engine preamble; the AllReduce's own
    doorbell (~37us local + up to ~20us core start skew) is what gates
    it, so the cheap d-sharded local compute (only ~18us of DVE work
    vs ~42us for a replicated-W sequence-sharded variant) keeps the
    whole pre-collective phase inside the skew+bootstrap window.
  - The post-AllReduce softmax uses gpsimd.partition_all_reduce for the
    cross-partition max/sum instead of PE-transpose chains: the tail is
    ~6 ops instead of 11, cutting ~5us of serial semaphore-latency.
  - Every core computes the identical softmax and writes the full
    output; the host takes core 0's copy.
  - The warm-up collective's sink DMA is pinned to the end of the
    schedule via tile_wait_until so the Tile scheduler cannot place it
    ahead of real work on the same engine queue (its wait on the warm
    AllGather would stall that queue for ~50us).
"""

import numpy as np

N_CORES = 8
SEQ = 32768
D = 200
H = 8192
DSH = D // N_CORES      # 25
P = 128
F = SEQ // P            # 256
KCH = H // P            # 64
NCH = 8                 # enc DMA / DVE chunks along F
FC = F // NCH           # 32


def build_kernel():
    import concourse.bacc as bacc
    import concourse.bass_isa as bass_isa
    import concourse.mybir as mybir
    import concourse.tile as tile

    fp32 = mybir.dt.float32
    nc = bacc.Bacc(
        "TRN2",
        target_bir_lowering=False,
        debug=False,
        num_devices=N_CORES,
    )

    # Host-prepacked layouts (see shard_inputs):
    #   encP [128, 256*25]: [p, f, d] with global s = p*256 + f
    #   wP   [128, 25*64]:  [p, d, k] with h = k*128 + p  (d-major!)
    #   hidP [128, 64]:     [p, k]    with h = k*128 + p
    encP = nc.dram_tensor("encP", [P, F * DSH], fp32, kind="ExternalInput")
    wP = nc.dram_tensor("wP", [P, DSH * KCH], fp32, kind="ExternalInput")
    hidP = nc.dram_tensor("hidP", [P, KCH], fp32, kind="ExternalInput")
    out = nc.dram_tensor("out", [SEQ], fp32, kind="ExternalOutput")
    # Sink for the warm-up collective (kept live so it isn't DCE'd).
    warm_out = nc.dram_tensor("warm_out", [2, 4], fp32,
                              kind="ExternalOutput")

    rg = [list(range(N_CORES))]

    with tile.TileContext(nc) as tc:
        with (
            tc.tile_pool(name="sb", bufs=1) as sb,
            tc.tile_pool(name="dram", bufs=1, space="DRAM") as dram,
        ):
            # ---- warm-up collective, FIRST and with NO data dependencies:
            # rings the runtime's collective doorbell immediately after the
            # fixed engine preamble so the entry barrier + first-collective
            # setup run while the DMAs/DVE work proceed. Pair groups: the
            # pairwise mesh completes faster than the 8-core one.
            warm_b = nc.inline_tensor(np.zeros((1, 4), np.float32),
                                      name="warm_src")
            warm_g = dram.tile([2, 4], fp32)
            nc.gpsimd.collective_compute(
                "AllGather",
                mybir.AluOpType.bypass,
                replica_groups=[[2 * i, 2 * i + 1] for i in range(N_CORES // 2)],
                ins=[warm_b.ap().opt()],
                outs=[warm_g[:].opt()],
            )

            # ---- loads (w + hid first: they gate the v chain; W split in
            # two so the DVE starts on the first half) ----
            w_sb = sb.tile([P, DSH * KCH], fp32)
            DH = 12                      # first v d-chunk
            wh = DH * KCH
            nc.sync.dma_start(w_sb[:, 0:wh], wP.ap()[:, 0:wh])
            h_sb = sb.tile([P, KCH], fp32)
            nc.sync.dma_start(h_sb[:], hidP.ap())
            nc.sync.dma_start(w_sb[:, wh:], wP.ap()[:, wh:])
            # All input DMAs stay on the Sync queue in consumption order:
            # a second DMA queue steals bandwidth from the W transfer that
            # gates the DVE (measured: W slowed 3x when enc ran parallel).
            enc_sb = sb.tile([P, F * DSH], fp32)
            for c in range(NCH):
                sl = slice(c * FC * DSH, (c + 1) * FC * DSH)
                nc.sync.dma_start(enc_sb[:, sl], encP.ap()[:, sl])

            # ---- v_i = W_i^T @ hidden: DVE mult + unit-stride reduce in
            # two d-chunks (pipelined with the W DMA halves), partition
            # collapse + broadcast in ONE gpsimd.partition_all_reduce ----
            w3 = w_sb[:].rearrange("p (d k) -> p d k", d=DSH)
            vtmp = sb.tile([P, DSH], fp32)
            for d0, d1 in ((0, DH), (DH, DSH)):
                dn = d1 - d0
                prod_w = sb.tile([P, dn * KCH], fp32, tag="prodw", bufs=2)
                h_b = (
                    h_sb[:]
                    .rearrange("p k -> p () k")
                    .broadcast_to([P, dn, KCH])
                )
                nc.vector.tensor_tensor(
                    out=prod_w[:].rearrange("p (d k) -> p d k", d=dn),
                    in0=w3[:, d0:d1, :],
                    in1=h_b,
                    op=mybir.AluOpType.mult,
                )
                nc.vector.reduce_sum(
                    vtmp[:, d0:d1],
                    prod_w[:].rearrange("p (d k) -> p d k", d=dn),
                    axis=mybir.AxisListType.X,
                )
            v_bc = sb.tile([P, DSH], fp32)
            nc.gpsimd.partition_all_reduce(
                v_bc[:], vtmp[:], channels=P, reduce_op=bass_isa.ReduceOp.add
            )

            # ---- partial energies e_i[p, f] = sum_d enc[p, f, d] * v[d];
            # each chunk's slice bounces to DRAM as soon as its reduce
            # lands so the last bounce overlaps the tail of the DVE ----
            bounce = dram.tile([P, F], fp32)
            esum = dram.tile([P, F], fp32, addr_space="Shared")
            e_part = sb.tile([P, F], fp32)
            for c in range(NCH):
                sl3 = enc_sb[:].rearrange("p (f d) -> p f d", d=DSH)[
                    :, c * FC : (c + 1) * FC, :
                ]
                eprod = sb.tile([P, FC * DSH], fp32, tag="eprod", bufs=2)
                v_b = (
                    v_bc[:]
                    .rearrange("p d -> p () d")
                    .broadcast_to([P, FC, DSH])
                )
                nc.vector.tensor_tensor(
                    out=eprod[:].rearrange("p (f d) -> p f d", d=DSH),
                    in0=sl3,
                    in1=v_b,
                    op=mybir.AluOpType.mult,
                )
                nc.vector.reduce_sum(
                    e_part[:, c * FC : (c + 1) * FC],
                    eprod[:].rearrange("p (f d) -> p f d", d=DSH),
                    axis=mybir.AxisListType.X,
                )
                nc.sync.dma_start(
                    bounce[:, c * FC : (c + 1) * FC],
                    e_part[:, c * FC : (c + 1) * FC],
                )
            nc.gpsimd.collective_compute(
                "AllReduce",
                mybir.AluOpType.add,
                replica_groups=rg,
                ins=[bounce[:].opt()],
                outs=[esum[:].opt()],
            )
            # e_sum comes back in two halves so the row-max of half 1
            # overlaps the DMA of half 2.
            e_sb = sb.tile([P, F], fp32)
            nc.sync.dma_start(e_sb[:, 0 : F // 2], esum[:, 0 : F // 2])
            nc.sync.dma_start(e_sb[:, F // 2 : F], esum[:, F // 2 : F])

            # ---- replicated softmax over [128, 256] via partition_all_reduce
            # (short serial chain, no PE transposes) ----
            m_h1 = sb.tile([P, 1], fp32)
            nc.vector.reduce_max(m_h1[:], e_sb[:, 0 : F // 2],
                                 axis=mybir.AxisListType.X)
            m_h2 = sb.tile([P, 1], fp32)
            nc.vector.reduce_max(m_h2[:], e_sb[:, F // 2 : F],
                                 axis=mybir.AxisListType.X)
            m_p = sb.tile([P, 1], fp32)
            nc.vector.tensor_tensor(m_p[:], m_h1[:], m_h2[:],
                                    op=mybir.AluOpType.max)
            M_bc = sb.tile([P, 1], fp32)
            nc.gpsimd.partition_all_reduce(
                M_bc[:], m_p[:], channels=P, reduce_op=bass_isa.ReduceOp.max
            )
            negM = sb.tile([P, 1], fp32)
            nc.vector.tensor_scalar_mul(negM[:], M_bc[:], -1.0)
            q = sb.tile([P, F], fp32)
            s_p = sb.tile([P, 1], fp32)
            nc.scalar.activation(
                q[:], e_sb[:], mybir.ActivationFunctionType.Exp,
                bias=negM[:], scale=1.0, accum_out=s_p[:],
            )
            S_bc = sb.tile([P, 1], fp32)
            nc.gpsimd.partition_all_reduce(
                S_bc[:], s_p[:], channels=P, reduce_op=bass_isa.ReduceOp.add
            )
            rS = sb.tile([P, 1], fp32)
            nc.vector.reciprocal(rS[:], S_bc[:])
            # scale + store in two halves so the first DMA overlaps the
            # second multiply
            o_sb = sb.tile([P, F], fp32)
            out2d = out.ap().rearrange("(p f) -> p f", p=P)
            nc.vector.tensor_scalar_mul(o_sb[:, 0 : F // 2],
                                        q[:, 0 : F // 2], rS[:])
            nc.sync.dma_start(out2d[:, 0 : F // 2], o_sb[:, 0 : F // 2])
            nc.vector.tensor_scalar_mul(o_sb[:, F // 2 : F],
                                        q[:, F // 2 : F], rS[:])
            nc.sync.dma_start(out2d[:, F // 2 : F], o_sb[:, F // 2 : F])

            # Keep the warm-up collective live. tile_wait_until pins it to
            # the end of the Tile scheduler's timeline so its wait on the
            # warm AllGather never stalls real work queued after it.
            with tc.tile_wait_until(1.0):
                nc.scalar.dma_start(warm_out.ap(), warm_g[:])

    nc.compile()
    return nc


def shard_inputs(hidden, encoder_outputs, W, b):
    hidden = np.asarray(hidden, dtype=np.float32)
    enc = np.asarray(encoder_outputs, dtype=np.float32)
    W = np.asarray(W, dtype=np.float32)
    enc3 = enc.reshape(P, F, D)          # s = p*F + f
    w3 = W.reshape(KCH, P, D)            # h = k*P + p
    hidP = np.ascontiguousarray(hidden.reshape(KCH, P).T)  # [p, k]
    in_maps = []
    for i in range(N_CORES):
        sl = slice(i * DSH, (i + 1) * DSH)
        encP_i = np.ascontiguousarray(enc3[:, :, sl]).reshape(P, F * DSH)
        wP_i = np.ascontiguousarray(
            w3[:, :, sl].transpose(1, 2, 0)       # [p, d, k]
        ).reshape(P, DSH * KCH)
        in_maps.append({"encP": encP_i, "wP": wP_i, "hidP": hidP})
    return in_maps


_NC_CACHE = {}


def _get_nc():
    if "nc" not in _NC_CACHE:
        _NC_CACHE["nc"] = build_kernel()
    return _NC_CACHE["nc"]


def kernel(hidden, encoder_outputs, W, b):
    from concourse import bass_utils

    nc = _get_nc()
    in_maps = shard_inputs(hidden, encoder_outputs, W, b)
    res = bass_utils.run_bass_kernel_spmd(
        nc, in_maps, core_ids=list(range(N_CORES))
    )
    out = np.asarray(res.results[0]["out"], dtype=np.float32)
    return out.reshape(1, 1, SEQ)


# revision 31
# speedup vs baseline: 1.0703x; 1.0703x over previous
"""Bass/Trainium2 kernel for nn_Attn (Bahdanau 'general' attention scoring).

Reference math:
    energies = einsum('sd,hd,h->s', enc, W, hidden) + b.hidden
    out      = softmax(energies)[None, None, :]

Factorization:
    v = W^T @ hidden (200-dim), energies = enc @ v (+ const; softmax cancels
    the constant b.hidden term, so b is dropped).

Distribution (8 NeuronCores, one TRN2 chip) — d-sharding: core i owns
d-slice [25*i, 25*(i+1)) of the contraction dim:
    W slice  [8192, 25]  -> v_i = W_i^T @ hidden (exact, local, no comm)
    enc slice [32768, 25] -> partial energies e_i[s] = enc[s, d_i] . v_i
for ALL 32768 positions, laid out [128, 256] (s = p*256 + f), then ONE
AllReduce(add) over the 128KB partials. Rationale from warmed profiles:
  - The collective entry barrier + first-collective setup complete by
    ~50us (global) when a dependency-free warm-up AllGather rings the
    doorbell right after the engine preamble; the AllReduce's own
    doorbell (~37us local + up to ~20us core start skew) is what gates
    it, so the cheap d-sharded local compute (only ~18us of DVE work
    vs ~42us for a replicated-W sequence-sharded variant) keeps the
    whole pre-collective phase inside the skew+bootstrap window.
  - The post-AllReduce softmax uses gpsimd.partition_all_reduce for the
    cross-partition max/sum instead of PE-transpose chains: the tail is
    ~6 ops instead of 11, cutting ~5us of serial semaphore-latency.
  - Every core computes the identical softmax and writes the full
    output; the host takes core 0's copy.
  - The warm-up collective's sink DMA is pinned to the end of the
    schedule via tile_wait_until so the Tile scheduler cannot place it
    ahead of real work on the same engine queue (its wait on the warm
    AllGather would stall that queue for ~50us).
"""

import numpy as np

N_CORES = 8
SEQ = 32768
D = 200
H = 8192
DSH = D // N_CORES      # 25
P = 128
F = SEQ // P            # 256
KCH = H // P            # 64
NCH = 4                 # enc DMA / DVE chunks along F
FC = F // NCH           # 64  (smaller chunks lose DVE efficiency: ~0.35us
                        #      fixed cost per DVE op dominates under 100K elems)


def build_kernel():
    import concourse.bacc as bacc
    import concourse.bass_isa as bass_isa
    import concourse.mybir as mybir
    import concourse.tile as tile

    fp32 = mybir.dt.float32
    nc = bacc.Bacc(
        "TRN2",
        target_bir_lowering=False,
        debug=False,
        num_devices=N_CORES,
    )

    # Host-prepacked layouts (see shard_inputs):
    #   encP [128, 256*25]: [p, f, d] with global s = p*256 + f
    #   wP   [128, 25*64]:  [p, d, k] with h = k*128 + p  (d-major!)
    #   hidP [128, 64]:     [p, k]    with h = k*128 + p
    encP = nc.dram_tensor("encP", [P, F * DSH], fp32, kind="ExternalInput")
    wP = nc.dram_tensor("wP", [P, DSH * KCH], fp32, kind="ExternalInput")
    hidP = nc.dram_tensor("hidP", [P, KCH], fp32, kind="ExternalInput")
    out = nc.dram_tensor("out", [SEQ], fp32, kind="ExternalOutput")
    # Sink for the warm-up collective (kept live so it isn't DCE'd).
    warm_out = nc.dram_tensor("warm_out", [2, 4], fp32,
                              kind="ExternalOutput")

    rg = [list(range(N_CORES))]

    with tile.TileContext(nc) as tc:
        with (
            tc.tile_pool(name="sb", bufs=1) as sb,
            tc.tile_pool(name="dram", bufs=1, space="DRAM") as dram,
        ):
            # ---- warm-up collective, FIRST and with NO data dependencies:
            # rings the runtime's collective doorbell immediately after the
            # fixed engine preamble so the entry barrier + first-collective
            # setup run while the DMAs/DVE work proceed. Pair groups: the
            # pairwise mesh completes faster than the 8-core one.
            warm_b = nc.inline_tensor(np.zeros((1, 4), np.float32),
                                      name="warm_src")
            warm_g = dram.tile([2, 4], fp32)
            nc.gpsimd.collective_compute(
                "AllGather",
                mybir.AluOpType.bypass,
                replica_groups=[[2 * i, 2 * i + 1] for i in range(N_CORES // 2)],
                ins=[warm_b.ap().opt()],
                outs=[warm_g[:].opt()],
            )

            # Dummy activation so the scalar engine's Exp ACT_TABLE_LOAD
            # (1.5us) happens here, not in front of the post-AllReduce exp.
            dummy = sb.tile([1, 1], fp32)
            nc.vector.memset(dummy[:], 0.0)
            dummy2 = sb.tile([1, 1], fp32)
            nc.scalar.activation(dummy2[:], dummy[:],
                                 mybir.ActivationFunctionType.Exp)

            # ---- loads (w + hid first: they gate the v chain; W split in
            # two so the DVE starts on the first half) ----
            w_sb = sb.tile([P, DSH * KCH], fp32)
            DH = 12                      # first v d-chunk
            wh = DH * KCH
            nc.sync.dma_start(w_sb[:, 0:wh], wP.ap()[:, 0:wh])
            h_sb = sb.tile([P, KCH], fp32)
            nc.sync.dma_start(h_sb[:], hidP.ap())
            nc.sync.dma_start(w_sb[:, wh:], wP.ap()[:, wh:])
            # All input DMAs stay on the Sync queue in consumption order:
            # a second DMA queue steals bandwidth from the W transfer that
            # gates the DVE (measured: W slowed 3x when enc ran parallel).
            enc_sb = sb.tile([P, F * DSH], fp32)
            for c in range(NCH):
                sl = slice(c * FC * DSH, (c + 1) * FC * DSH)
                nc.sync.dma_start(enc_sb[:, sl], encP.ap()[:, sl])

            # ---- v_i = W_i^T @ hidden: DVE mult + unit-stride reduce in
            # two d-chunks (pipelined with the W DMA halves), partition
            # collapse + broadcast in ONE gpsimd.partition_all_reduce ----
            w3 = w_sb[:].rearrange("p (d k) -> p d k", d=DSH)
            vtmp = sb.tile([P, DSH], fp32)
            for d0, d1 in ((0, DH), (DH, DSH)):
                dn = d1 - d0
                prod_w = sb.tile([P, dn * KCH], fp32, tag="prodw", bufs=2)
                h_b = (
                    h_sb[:]
                    .rearrange("p k -> p () k")
                    .broadcast_to([P, dn, KCH])
                )
                nc.vector.tensor_tensor(
                    out=prod_w[:].rearrange("p (d k) -> p d k", d=dn),
                    in0=w3[:, d0:d1, :],
                    in1=h_b,
                    op=mybir.AluOpType.mult,
                )
                nc.vector.reduce_sum(
                    vtmp[:, d0:d1],
                    prod_w[:].rearrange("p (d k) -> p d k", d=dn),
                    axis=mybir.AxisListType.X,
                )
            v_bc = sb.tile([P, DSH], fp32)
            nc.gpsimd.partition_all_reduce(
                v_bc[:], vtmp[:], channels=P, reduce_op=bass_isa.ReduceOp.add
            )

            # ---- partial energies e_i[p, f] = sum_d enc[p, f, d] * v[d];
            # each chunk's slice bounces to DRAM as soon as its reduce
            # lands so the last bounce overlaps the tail of the DVE ----
            bounce = dram.tile([P, F], fp32)
            esum = dram.tile([P, F], fp32, addr_space="Shared")
            e_part = sb.tile([P, F], fp32)
            for c in range(NCH):
                sl3 = enc_sb[:].rearrange("p (f d) -> p f d", d=DSH)[
                    :, c * FC : (c + 1) * FC, :
                ]
                eprod = sb.tile([P, FC * DSH], fp32, tag="eprod", bufs=2)
                v_b = (
                    v_bc[:]
                    .rearrange("p d -> p () d")
                    .broadcast_to([P, FC, DSH])
                )
                nc.vector.tensor_tensor(
                    out=eprod[:].rearrange("p (f d) -> p f d", d=DSH),
                    in0=sl3,
                    in1=v_b,
                    op=mybir.AluOpType.mult,
                )
                nc.vector.reduce_sum(
                    e_part[:, c * FC : (c + 1) * FC],
                    eprod[:].rearrange("p (f d) -> p f d", d=DSH),
                    axis=mybir.AxisListType.X,
                )
                nc.sync.dma_start(
                    bounce[:, c * FC : (c + 1) * FC],
                    e_part[:, c * FC : (c + 1) * FC],
                )
            nc.gpsimd.collective_compute(
                "AllReduce",
                mybir.AluOpType.add,
                replica_groups=rg,
                ins=[bounce[:].opt()],
                outs=[esum[:].opt()],
            )
            # e_sum comes back in two halves so the row-max of half 1
            # overlaps the DMA of half 2.
            e_sb = sb.tile([P, F], fp32)
            nc.sync.dma_start(e_sb[:, 0 : F // 2], esum[:, 0 : F // 2])
            nc.sync.dma_start(e_sb[:, F // 2 : F], esum[:, F // 2 : F])

            # ---- replicated softmax over [128, 256] via partition_all_reduce
            # (short serial chain, no PE transposes) ----
            m_h1 = sb.tile([P, 1], fp32)
            nc.vector.reduce_max(m_h1[:], e_sb[:, 0 : F // 2],
                                 axis=mybir.AxisListType.X)
            m_h2 = sb.tile([P, 1], fp32)
            nc.vector.reduce_max(m_h2[:], e_sb[:, F // 2 : F],
                                 axis=mybir.AxisListType.X)
            m_p = sb.tile([P, 1], fp32)
            nc.vector.tensor_tensor(m_p[:], m_h1[:], m_h2[:],
                                    op=mybir.AluOpType.max)
            M_bc = sb.tile([P, 1], fp32)
            nc.gpsimd.partition_all_reduce(
                M_bc[:], m_p[:], channels=P, reduce_op=bass_isa.ReduceOp.max
            )
            negM = sb.tile([P, 1], fp32)
            nc.vector.tensor_scalar_mul(negM[:], M_bc[:], -1.0)
            q = sb.tile([P, F], fp32)
            s_p = sb.tile([P, 1], fp32)
            nc.scalar.activation(
                q[:], e_sb[:], mybir.ActivationFunctionType.Exp,
                bias=negM[:], scale=1.0, accum_out=s_p[:],
            )
            S_bc = sb.tile([P, 1], fp32)
            nc.gpsimd.partition_all_reduce(
                S_bc[:], s_p[:], channels=P, reduce_op=bass_isa.ReduceOp.add
            )
            rS = sb.tile([P, 1], fp32)
            nc.vector.reciprocal(rS[:], S_bc[:])
            # scale + store in two halves so the first DMA overlaps the
            # second multiply
            o_sb = sb.tile([P, F], fp32)
            out2d = out.ap().rearrange("(p f) -> p f", p=P)
            nc.vector.tensor_scalar_mul(o_sb[:, 0 : F // 2],
                                        q[:, 0 : F // 2], rS[:])
            nc.sync.dma_start(out2d[:, 0 : F // 2], o_sb[:, 0 : F // 2])
            nc.vector.tensor_scalar_mul(o_sb[:, F // 2 : F],
                                        q[:, F // 2 : F], rS[:])
            nc.sync.dma_start(out2d[:, F // 2 : F], o_sb[:, F // 2 : F])

            # Keep the warm-up collective live. tile_wait_until pins the
            # sink DMA to the end of the scheduler's timeline; on the
            # gpsimd queue even a misplaced copy only costs issue time,
            # since every gpsimd op after the warm AllGather completes
            # (~50us) runs much later anyway.
            with tc.tile_wait_until(1.0):
                nc.gpsimd.dma_start(warm_out.ap(), warm_g[:])

    nc.compile()
    return nc


def shard_inputs(hidden, encoder_outputs, W, b):
    hidden = np.asarray(hidden, dtype=np.float32)
    enc = np.asarray(encoder_outputs, dtype=np.float32)
    W = np.asarray(W, dtype=np.float32)
    enc3 = enc.reshape(P, F, D)          # s = p*F + f
    w3 = W.reshape(KCH, P, D)            # h = k*P + p
    hidP = np.ascontiguousarray(hidden.reshape(KCH, P).T)  # [p, k]
    in_maps = []
    for i in range(N_CORES):
        sl = slice(i * DSH, (i + 1) * DSH)
        encP_i = np.ascontiguousarray(enc3[:, :, sl]).reshape(P, F * DSH)
        wP_i = np.ascontiguousarray(
            w3[:, :, sl].transpose(1, 2, 0)       # [p, d, k]
        ).reshape(P, DSH * KCH)
        in_maps.append({"encP": encP_i, "wP": wP_i, "hidP": hidP})
    return in_maps


_NC_CACHE = {}


def _get_nc():
    if "nc" not in _NC_CACHE:
        _NC_CACHE["nc"] = build_kernel()
    return _NC_CACHE["nc"]


def kernel(hidden, encoder_outputs, W, b):
    from concourse import bass_utils

    nc = _get_nc()
    in_maps = shard_inputs(hidden, encoder_outputs, W, b)
    res = bass_utils.run_bass_kernel_spmd(
        nc, in_maps, core_ids=list(range(N_CORES))
    )
    out = np.asarray(res.results[0]["out"], dtype=np.float32)
    return out.reshape(1, 1, SEQ)
